# revision 33
# baseline (speedup 1.0000x reference)
"""Chunked cross-attention (RETRO-style) Trainium2 Bass kernel.

Problem shapes (hardcoded):
  h: [4, 1024, 1024] f32, e: [4, 16, 2, 128, 1024] f32
  D_MODEL=1024, N_HEADS=16, D_K=64, CHUNK_LEN=64, B=4, C=16, N=2, NL=128

Sharding: 8 cores = batch(4) x chunk-group(2). Chunks are independent
end-to-end (each chunk's queries attend only to its own neighbors, and the
output projection is per-position), so there are no collectives.

Per-core kernel (matmul operands fp8 DoubleRow where possible, f32 PSUM):
  - Host packs all inputs into 5 partition-major dram tensors (one DMA issue
    each, ordered by first use) so the PE starts ~5us in instead of ~16us.
  - RMSNorm in transposed space (sum of squares via ones-matmul); rsqrt via
    reciprocal_approx_fast + Sqrt activation.
  - Q^T / K^T with weight blocks stationary; V natural with e^T stationary.
  - Scores per (chunk, head): S[i, j'] with two chunks packed on partitions.
  - exp on ScalarE, row-sums + normalize on VectorE, xbar-DMA transpose of the
    normalized attention, attn @ V accumulated over j' blocks, then the output
    projection (fp8 DoubleRow, oT stationary) with f32 residual add.
  - Output stores issued from the GpSimd software DGE queue.
"""

import os
import numpy as np
import ml_dtypes

import concourse.bass as bass
import concourse.bacc as bacc
import concourse.mybir as mybir
import concourse.tile as tile
from concourse.bass_utils import run_bass_kernel_spmd

BF16 = mybir.dt.bfloat16
F32 = mybir.dt.float32
F8 = mybir.dt.float8e4
DR = mybir.MatmulPerfMode.DoubleRow
AF = mybir.ActivationFunctionType

P = 128
D = 1024       # d_model
HD = 1024      # n_heads * d_k
NH = 16        # heads
DK = 64
CL = 64        # chunk len
NCH = 8        # chunks per core
JC = 256       # kv rows per chunk (n * nl)
JP = 512       # kv rows per chunk-pair
NPAIR = 4      # chunk pairs per core
I = 512        # q rows per core
DB = D // P    # 8 d blocks
HB = HD // P   # 8 hd blocks
EPS = 1e-8

_CACHED = {}


def _build_nc(with_bq=False):
    nc = bacc.Bacc("TRN2", target_bir_lowering=False, debug=False)

    # packed inputs, partition-major, one DMA each, ordered by first use:
    #  pk0: hT [p, 8db, 512i] | wq [p, 4blk, 2, 1024]
    #  pk1a: wk [p, 4blk, 2, 1024] | eT pairs 0-1 [p, 2pr, 8db, 512]
    #  pk2: wv [p, 4blk, 2, 1024] | wo [p, 4blk, 2, 1024]
    #  pk1b: eT pairs 2-3
    #  pk3 (f32): hres [p, 4pr, 1024] | bqt [p, 8]
    pk0a = nc.dram_tensor("pk0a", [P, 4096 + 4096], F8, kind="ExternalInput").ap()
    pk0b = nc.dram_tensor("pk0b", [P, 4096], F8, kind="ExternalInput").ap()
    pk1a = nc.dram_tensor("pk1a", [P, 8192 + 8192], F8, kind="ExternalInput").ap()
    pk2 = nc.dram_tensor("pk2", [P, 8192 + 8192], F8, kind="ExternalInput").ap()
    pk1b = nc.dram_tensor("pk1b", [P, 8192], F8, kind="ExternalInput").ap()
    pk3 = nc.dram_tensor("pk3", [P, 4096 + 8], BF16, kind="ExternalInput").ap()
    out = nc.dram_tensor("out", [I, D], BF16, kind="ExternalOutput").ap()

    with tile.TileContext(nc) as tc:
        _emit(nc, tc, pk0a, pk0b, pk1a, pk2, pk1b, pk3, out, with_bq)
    nc.compile()
    return nc


def _emit(nc, tc, pk0a, pk0b, pk1a, pk2, pk1b, pk3, out, with_bq=False):
    WITH_BQ = with_bq
    from contextlib import ExitStack

    with ExitStack() as ctx:
        const = ctx.enter_context(tc.tile_pool(name="const", bufs=1))
        persist = ctx.enter_context(tc.tile_pool(name="persist", bufs=1))
        sqp = ctx.enter_context(tc.tile_pool(name="sq", bufs=2))
        ktp = ctx.enter_context(tc.tile_pool(name="ktp", bufs=2))
        vp = ctx.enter_context(tc.tile_pool(name="vp", bufs=2))
        esp = ctx.enter_context(tc.tile_pool(name="esp", bufs=2))
        estp = ctx.enter_context(tc.tile_pool(name="estp", bufs=2))
        sump = ctx.enter_context(tc.tile_pool(name="sump", bufs=2))
        otp = ctx.enter_context(tc.tile_pool(name="otp", bufs=2))
        outp = ctx.enter_context(tc.tile_pool(name="outp", bufs=2))
        psA = ctx.enter_context(tc.tile_pool(name="psA", bufs=4, space="PSUM"))
        psS = ctx.enter_context(tc.tile_pool(name="psS", bufs=2, space="PSUM"))
        psO = ctx.enter_context(tc.tile_pool(name="psO", bufs=2, space="PSUM"))

        # ---- packed SBUF landing tiles ----
        pk0a_sb = const.tile([P, 4096 + 4096], F8, name="pk0a_sb")
        pk0b_sb = const.tile([P, 4096], F8, name="pk0b_sb")
        pk1a_sb = const.tile([P, 8192 + 8192], F8, name="pk1a_sb")
        pk2_sb = const.tile([P, 8192 + 8192], F8, name="pk2_sb")
        pk1b_sb = const.tile([P, 8192], F8, name="pk1b_sb")
        pk3_sb = const.tile([P, 4096 + 8], BF16, name="pk3_sb")

        hT = pk0a_sb[:, 0:4096].rearrange("p (db i) -> p db i", db=DB)
        # wq split by output-column half so Q(hb 0-3) starts after 1MB
        wqA_v = pk0a_sb[:, 4096:8192].rearrange(
            "p (blk two h) -> p blk two h", blk=4, two=2)
        wqB_v = pk0b_sb[:, 0:4096].rearrange(
            "p (blk two h) -> p blk two h", blk=4, two=2)
        wk_v = pk1a_sb[:, 0:8192].rearrange(
            "p (blk two h) -> p blk two h", blk=4, two=2)
        eT01 = pk1a_sb[:, 8192:16384].rearrange(
            "p (pr db j) -> p pr db j", pr=2, db=DB)
        eT23 = pk1b_sb[:, 0:8192].rearrange(
            "p (pr db j) -> p pr db j", pr=2, db=DB)
        wv_v = pk2_sb[:, 0:8192].rearrange(
            "p (blk two h) -> p blk two h", blk=4, two=2)
        wo_v = pk2_sb[:, 8192:16384].rearrange(
            "p (blk two d) -> p blk two d", blk=4, two=2)
        hres_v = pk3_sb[:, 0:4096].rearrange("p (pr d) -> p pr d", pr=NPAIR)
        bq_sb = pk3_sb[:, 4096:4104]

        def eT(p):
            return eT01[:, p, :, :] if p < 2 else eT23[:, p - 2, :, :]

        ones = const.tile([P, 1], BF16, name="ones")
        ones_row = const.tile([1, P], BF16, name="ones_row")
        zeros = const.tile([P, 1], F32, name="zeros")
        epsc = const.tile([1, 1], F32, name="epsc")
        qT = persist.tile([P, HB, I], BF16, name="qT")

        kT = [None] * NPAIR
        v = [None] * NPAIR
        expS = [None] * NPAIR
        expST = [None] * NPAIR
        oT = [None] * NPAIR

        def emit_kt(p):
            # K^T [hd, j'] : weight blocks stationary
            kT[p] = ktp.tile([P, HB, JP], BF16, tag="kT", name=f"kT{p}")
            for hb in range(HB):
                ps_k = psA.tile([P, JP], F32, tag="A")
                for blk in range(4):
                    nc.tensor.matmul(
                        ps_k[:],
                        wk_v[:, blk, :, hb * P:(hb + 1) * P],
                        eT(p)[:, 2 * blk:2 * blk + 2, :],
                        start=(blk == 0),
                        stop=(blk == 3),
                        perf_mode=DR,
                    )
                nc.scalar.copy(kT[p][:, hb, :], ps_k[:])

        def emit_v(p):
            # V [j', hd] : e^T blocks stationary.  PSUM evacuation on DVE
            # mid-kernel (ScalarE queue backs up behind exp/oT ACTs at pair
            # boundaries); on ScalarE for the last pair (DVE runs the
            # endgame softmax chain there).
            v[p] = vp.tile([P, 4, HD], BF16, tag="v", name=f"v{p}")
            for jb in range(4):
                for half in range(2):
                    ps_v = psA.tile([P, 512], F32, tag="A")
                    for blk in range(4):
                        nc.tensor.matmul(
                            ps_v[:],
                            eT(p)[:, 2 * blk:2 * blk + 2, jb * P:(jb + 1) * P],
                            wv_v[:, blk, :, half * 512:(half + 1) * 512],
                            start=(blk == 0),
                            stop=(blk == 3),
                            perf_mode=DR,
                        )
                    nc.scalar.copy(
                        v[p][:, jb, half * 512:(half + 1) * 512], ps_v[:])

        def emit_S(p):
            # scores + exp; psS partition layout (hpar, i) so concurrent
            # row-group pairs write different output partitions (same-bank
            # same-partition concurrent PE writes are a HW fault).
            expS[p] = esp.tile([P, HB, JP], BF16, tag="expS", name=f"expS{p}")
            expST[p] = estp.tile([P, 4 * HB, P], BF16, tag="expST", name=f"expST{p}")
            sums = sump.tile([P, NH], F32, tag="sums")
            recip = sump.tile([P, NH], F32, tag="recip")
            for s in range(HB):  # head pair s -> heads 2s, 2s+1
                ps_s = psS.tile([P, 512], F32)
                for hpar in range(2):
                    for c01 in range(2):
                        nc.tensor.matmul(
                            ps_s[64 * hpar:64 * hpar + 64, 256 * c01:256 * c01 + 256],
                            qT[64 * hpar:64 * hpar + 64, s,
                               (2 * p + c01) * CL:(2 * p + c01) * CL + CL],
                            kT[p][64 * hpar:64 * hpar + 64, s,
                                  c01 * JC:(c01 + 1) * JC],
                            start=True, stop=True,
                        )
                nc.scalar.activation(
                    expS[p][:, s, :], ps_s[:], AF.Exp, bias=zeros[:],
                )
                nc.vector.reduce_sum(
                    sums[:, 2 * s:2 * s + 2],
                    expS[p][:, s, :].rearrange("p (c j) -> p c j", c=2),
                    axis=mybir.AxisListType.X)
                nc.vector.reciprocal_approx_fast(
                    recip[:, 2 * s:2 * s + 2], sums[:, 2 * s:2 * s + 2])
                nc.vector.tensor_mul(
                    expS[p][:, s, :].rearrange("p (c j) -> p c j", c=2),
                    expS[p][:, s, :].rearrange("p (c j) -> p c j", c=2),
                    recip[:, 2 * s:2 * s + 2].unsqueeze(-1).broadcast_to(
                        [P, 2, JC]))
                if s % 4 == 3:
                    # xbar transpose of the 4 finished s-tiles:
                    # out[pp, t, r] = attn[r, t*128+pp]
                    # alternate queues so a pair's two transposes overlap;
                    # last pair keeps both on sync (idle then) -- a scalar
                    # issue would queue behind the endgame exp/copy ACTs
                    eng = nc.sync if s == 3 else nc.scalar
                    eng.dma_start(
                        out=expST[p][:, 4 * (s - 3):4 * (s - 3) + 16, :],
                        in_=expS[p][:, s - 3:s + 1, :].rearrange(
                            "p a b -> p (a b)"),
                        transpose=True,
                    )

        def emit_o(p):
            # o^T = attn @ V  (V slices stationary, attn^T streaming)
            oT[p] = otp.tile([P, HB, P], F8, tag="oT", name=f"oT{p}")
            for t2 in range(2):
                po = psO.tile([P, 512], F32)
                for c01 in range(2):
                    for kk in range(4):
                        for hpar in range(2):
                            s = 4 * t2 + kk
                            h = 2 * s + hpar
                            slot = c01 * 4 + kk
                            for jb in range(2):
                                nc.tensor.matmul(
                                    po[64 * hpar:64 * hpar + 64,
                                       64 * slot:64 * slot + 64],
                                    v[p][:, c01 * 2 + jb, h * DK:(h + 1) * DK],
                                    expST[p][:, 4 * s + 2 * c01 + jb,
                                             64 * hpar:64 * hpar + 64],
                                    start=(jb == 0), stop=(jb == 1),
                                )
                for c01 in range(2):
                    nc.scalar.copy(
                        oT[p][:, 4 * t2:4 * t2 + 4, 64 * c01:64 * c01 + 64],
                        po[:, 256 * c01:256 * c01 + 256].rearrange(
                            "p (a b) -> p a b", a=4
                        ),
                    )

        def emit_outproj(p):
            # out = oT^T @ wo + hres   (oT stationary fp8 DoubleRow)
            # last pair in quarter-column chunks so add+store pipeline and
            # the final store is small
            nchunk, w = (4, 256) if p == NPAIR - 1 else (2, 512)
            for ch in range(nchunk):
                ps_o = psA.tile([P, w], F32, tag="A")
                for blk in range(4):
                    nc.tensor.matmul(
                        ps_o[:],
                        oT[p][:, 2 * blk:2 * blk + 2, :],
                        wo_v[:, blk, :, ch * w:(ch + 1) * w],
                        start=(blk == 0),
                        stop=(blk == 3),
                        perf_mode=DR,
                    )
                osb = outp.tile([P, w], BF16, tag=f"osb{w}", bufs=4)
                nc.vector.tensor_add(
                    osb[:],
                    ps_o[:],
                    hres_v[:, p, ch * w:(ch + 1) * w],
                )
                # last pair's stores on the (idle by then) sync queue: the
                # gpsimd software DGE issues slowly and pays a ~2.5us drain
                eng = nc.sync if p == NPAIR - 1 else nc.gpsimd
                eng.dma_start(
                    out[p * P:(p + 1) * P, ch * w:(ch + 1) * w],
                    osb[:])

        # ---- prologue ----
        nc.vector.memset(ones[:], 1.0)
        nc.vector.memset(ones_row[:], 1.0)
        nc.vector.memset(zeros[:], 0.0)
        nc.vector.memset(epsc[:], EPS)
        # prewarm ScalarE LUTs (Exp/Sqrt/Square table loads ~1.3us on first use)
        warm = const.tile([1, 3], F32, name="warm")
        nc.scalar.activation(warm[:, 0:1], epsc[:], AF.Exp, bias=zeros[0:1, :])
        nc.scalar.activation(warm[:, 1:2], epsc[:], AF.Sqrt, bias=zeros[0:1, :])
        nc.scalar.activation(warm[:, 2:3], epsc[:], AF.Square, bias=zeros[0:1, :])
        # packed input DMAs, one per dram tensor, in consumption order
        nc.sync.dma_start(pk0a_sb[:], pk0a)
        nc.sync.dma_start(pk0b_sb[:], pk0b)
        nc.sync.dma_start(pk1a_sb[:], pk1a)
        nc.sync.dma_start(pk2_sb[:], pk2)
        nc.sync.dma_start(pk1b_sb[:], pk1b)
        nc.sync.dma_start(pk3_sb[:], pk3)

        # rms squares on ScalarE (DVE stays free for the qT muls later)
        sq = [None] * DB
        for db in range(DB):
            sq[db] = sqp.tile([P, I], BF16, tag="sq", bufs=8, name=f"sq{db}")
            nc.scalar.activation(sq[db][:], hT[:, db, :], AF.Square,
                                 bias=zeros[:])

        # ---- Q^T from raw hT; rmsnorm scale applied at the epilogue ----
        qTraw = persist.tile([P, HB, I], BF16, name="qTraw")
        for hb in range(HB):
            wq_v, hbo = (wqA_v, hb) if hb < 4 else (wqB_v, hb - 4)
            ps_q = psA.tile([P, I], F32, tag="A")
            for blk in range(4):
                nc.tensor.matmul(
                    ps_q[:],
                    wq_v[:, blk, :, hbo * P:(hbo + 1) * P],
                    hT[:, 2 * blk:2 * blk + 2, :],
                    start=(blk == 0),
                    stop=(blk == 3),
                    perf_mode=DR,
                )
            nc.scalar.copy(qTraw[:, hb, :], ps_q[:])
            if hb == 3:
                # rmsnorm stats early so rstd is ready well before S(0)
                ps_ss = psA.tile([1, I], F32, tag="A")
                for db in range(DB):
                    nc.tensor.matmul(
                        ps_ss[:], ones[:], sq[db][:],
                        start=(db == 0), stop=(db == DB - 1)
                    )

        ms = persist.tile([1, I], BF16, name="ms")
        nc.scalar.activation(ms[:], ps_ss[:], AF.Identity, bias=epsc[:],
                             scale=1.0 / D)
        # broadcast ms across partitions (PE outer product, K=1, bf16 so a
        # single matmul pass), then rsqrt at full width via
        # reciprocal_approx_fast + Sqrt activation.
        ps_msb = psA.tile([P, I], F32, tag="A")
        nc.tensor.matmul(ps_msb[:], ones_row[:], ms[:], start=True, stop=True)
        inv_msf = persist.tile([P, I], F32, name="inv_msf")
        nc.vector.reciprocal_approx_fast(inv_msf[:], ps_msb[:])
        rstd_full = persist.tile([P, I], F32, name="rstd_full")
        # rstd/8 in one shot: sqrt(inv_ms / 64) (folds the attention scale)
        nc.scalar.activation(rstd_full[:], inv_msf[:], AF.Sqrt, bias=zeros[:],
                             scale=1.0 / 64.0)
        for hb in range(HB):
            # qT = qTraw * rstd/8  (column-wise); bq added after if nonzero
            nc.vector.tensor_mul(qT[:, hb, :], qTraw[:, hb, :], rstd_full[:])
            if WITH_BQ:
                nc.scalar.activation(
                    qT[:, hb, :], qT[:, hb, :], AF.Identity,
                    bias=bq_sb[:, hb:hb + 1], scale=1.0,
                )

        # ---- software-pipelined pair loop ----
        # PE stream: Q, rms, KT0, V0, S0, KT1, V1, o0, S1, op0, KT2, V2, o1,
        # S2, op1, KT3, S3, o2, V3, op2, o3, op3.  S(p+1) is scheduled right
        # after KT(p+1); for the last pair S3 runs before o2/V3 so its softmax
        # + transpose chain is covered by ~12us of PE work.
        emit_kt(0)
        emit_v(0)
        emit_S(0)
        for p in range(NPAIR):
            if p + 1 < NPAIR:
                emit_kt(p + 1)
                if p + 1 == NPAIR - 1:
                    emit_S(p + 1)
                else:
                    emit_v(p + 1)
            emit_o(p)
            if p + 1 < NPAIR:
                if p + 1 == NPAIR - 1:
                    emit_v(p + 1)
                else:
                    emit_S(p + 1)
            emit_outproj(p)


def _get_nc(with_bq=False):
    if with_bq not in _CACHED:
        _CACHED[with_bq] = _build_nc(with_bq)
    return _CACHED[with_bq]


def _make_in_maps(h, e, g_norm, Wq, bq, Wk, bk, Wv, bv, Wo, bo):
    f8 = ml_dtypes.float8_e4m3
    bf = ml_dtypes.bfloat16
    h = np.asarray(h, np.float32)
    e = np.asarray(e, np.float32)
    # fold g_norm into Wq (rmsnorm gain only feeds the q projection)
    wq_f = np.asarray(g_norm, np.float32)[:, None] * np.asarray(Wq, np.float32)

    def wlayout(w):
        # [1024, X] -> [p, blk, two, X] flat, row = blk*256 + two*128 + p
        return np.ascontiguousarray(
            w.reshape(4, 2, 128, -1).transpose(2, 0, 1, 3).reshape(128, -1))

    wq8 = wlayout(wq_f.astype(f8))
    wqA = np.ascontiguousarray(
        wq8.reshape(128, 4, 2, 1024)[:, :, :, :512].reshape(128, 4096))
    wqB = np.ascontiguousarray(
        wq8.reshape(128, 4, 2, 1024)[:, :, :, 512:].reshape(128, 4096))
    wk8 = wlayout(np.asarray(Wk, np.float32).astype(f8))
    wv8 = wlayout(np.asarray(Wv, np.float32).astype(f8))
    wo8 = wlayout(np.asarray(Wo, np.float32).astype(f8))
    # bq applied on device (pre-scaled by attention scale); bk is a no-op
    # through softmax; bv/bo fold into the residual below.
    bqt = (np.asarray(bq, np.float32) / 8.0).reshape(HB, P).T.astype(bf)
    out_bias = None
    bv = np.asarray(bv, np.float32)
    bo = np.asarray(bo, np.float32)
    if np.any(bv) or np.any(bo):
        out_bias = bv @ np.asarray(Wo, np.float32) + bo

    in_maps = []
    meta = []
    for b in range(4):
        for g in range(2):
            start = 63 + 512 * g
            stop = min(1024, start + 512)
            nvalid = stop - start
            hs = np.zeros((512, D), np.float32)
            hs[:nvalid] = h[b, start:stop]
            hres = hs if out_bias is None else hs + out_bias[None, :]
            # hT [p, db, i] flat: row d = db*128 + p
            hT8 = hs.T.astype(f8).reshape(8, 128, 512).transpose(
                1, 0, 2).reshape(128, 4096)
            es = e[b, 8 * g:8 * (g + 1)].reshape(NCH * JC, D)
            # eT [p, pr, db, 512]: row d = db*128 + p, col j = pr*512 + c
            et = np.ascontiguousarray(
                es.T.astype(f8).reshape(8, 128, 4, 512).transpose(
                    1, 2, 0, 3).reshape(128, 16384))
            hres_p = np.ascontiguousarray(
                hres.reshape(4, 128, 1024).transpose(1, 0, 2).reshape(
                    128, 4096)).astype(bf)
            in_maps.append({
                "pk0a": np.concatenate([hT8, wqA], axis=1),
                "pk0b": wqB,
                "pk1a": np.concatenate([wk8, et[:, :8192]], axis=1),
                "pk2": np.concatenate([wv8, wo8], axis=1),
                "pk1b": np.ascontiguousarray(et[:, 8192:]),
                "pk3": np.concatenate([hres_p, bqt], axis=1),
            })
            meta.append((b, start, nvalid))
    return in_maps, meta


def _assemble(h, results, meta):
    outf = np.array(h, np.float32, copy=True)
    for core, (b, start, nvalid) in enumerate(meta):
        outf[b, start:start + nvalid] = results[core]["out"][:nvalid].astype(
            np.float32)
    # rows [0, 63) stay h (zero-padded attention output region)
    return outf


def kernel(h, e, g_norm, Wq, bq, Wk, bk, Wv, bv, Wo, bo):
    in_maps, meta = _make_in_maps(h, e, g_norm, Wq, bq, Wk, bk, Wv, bv, Wo, bo)
    nc = _get_nc(bool(np.any(np.asarray(bq))))
    res = run_bass_kernel_spmd(nc, in_maps, list(range(8)))
    return _assemble(h, res.results, meta)


def kernel_timed(trace=True, **inputs):
    """test-harness entry: returns (output, exec_time_ns)."""
    in_maps, meta = _make_in_maps(**inputs)
    nc = _get_nc(bool(np.any(np.asarray(inputs["bq"]))))
    res = run_bass_kernel_spmd(nc, in_maps, list(range(8)), trace=trace)
    return _assemble(inputs["h"], res.results, meta), res.exec_time_ns


# revision 35
# speedup vs baseline: 1.0500x; 1.0500x over previous
"""Chunked cross-attention (RETRO-style) Trainium2 Bass kernel.

Problem shapes (hardcoded):
  h: [4, 1024, 1024] f32, e: [4, 16, 2, 128, 1024] f32
  D_MODEL=1024, N_HEADS=16, D_K=64, CHUNK_LEN=64, B=4, C=16, N=2, NL=128

Sharding: 8 cores = batch(4) x chunk-group(2). Chunks are independent
end-to-end (each chunk's queries attend only to its own neighbors, and the
output projection is per-position), so there are no collectives.

Per-core kernel (matmul operands fp8 DoubleRow where possible, f32 PSUM):
  - Host packs all inputs into 5 partition-major dram tensors (one DMA issue
    each, ordered by first use) so the PE starts ~5us in instead of ~16us.
  - RMSNorm in transposed space (sum of squares via ones-matmul); rsqrt via
    reciprocal_approx_fast + Sqrt activation.
  - Q^T / K^T with weight blocks stationary; V natural with e^T stationary.
  - Scores per (chunk, head): S[i, j'] with two chunks packed on partitions.
  - exp on ScalarE, row-sums + normalize on VectorE, xbar-DMA transpose of the
    normalized attention, attn @ V accumulated over j' blocks, then the output
    projection (fp8 DoubleRow, oT stationary) with f32 residual add.
  - Output stores issued from the GpSimd software DGE queue.
"""

import os
import numpy as np
import ml_dtypes

import concourse.bass as bass
import concourse.bacc as bacc
import concourse.mybir as mybir
import concourse.tile as tile
from concourse.bass_utils import run_bass_kernel_spmd

BF16 = mybir.dt.bfloat16
F32 = mybir.dt.float32
F8 = mybir.dt.float8e4
DR = mybir.MatmulPerfMode.DoubleRow
AF = mybir.ActivationFunctionType

P = 128
D = 1024       # d_model
HD = 1024      # n_heads * d_k
NH = 16        # heads
DK = 64
CL = 64        # chunk len
NCH = 8        # chunks per core
JC = 256       # kv rows per chunk (n * nl)
JP = 512       # kv rows per chunk-pair
NPAIR = 4      # chunk pairs per core
I = 512        # q rows per core
DB = D // P    # 8 d blocks
HB = HD // P   # 8 hd blocks
EPS = 1e-8

_CACHED = {}


def _build_nc(with_bq=False):
    nc = bacc.Bacc("TRN2", target_bir_lowering=False, debug=False)

    # packed inputs, partition-major, one DMA each, ordered by first use:
    #  pk0: hT [p, 8db, 512i] | wq [p, 4blk, 2, 1024]
    #  pk1a: wk [p, 4blk, 2, 1024] | eT pairs 0-1 [p, 2pr, 8db, 512]
    #  pk2: wv [p, 4blk, 2, 1024] | wo [p, 4blk, 2, 1024]
    #  pk1b: eT pairs 2-3
    #  pk3 (f32): hres [p, 4pr, 1024] | bqt [p, 8]
    pk0a = nc.dram_tensor("pk0a", [P, 4096 + 4096], F8, kind="ExternalInput").ap()
    pk0b = nc.dram_tensor("pk0b", [P, 4096], F8, kind="ExternalInput").ap()
    pk1a = nc.dram_tensor("pk1a", [P, 8192 + 8192], F8, kind="ExternalInput").ap()
    pk2 = nc.dram_tensor("pk2", [P, 8192 + 8192], F8, kind="ExternalInput").ap()
    pk1b = nc.dram_tensor("pk1b", [P, 8192], F8, kind="ExternalInput").ap()
    pk3 = nc.dram_tensor("pk3", [P, 4096 + 8], BF16, kind="ExternalInput").ap()
    out = nc.dram_tensor("out", [I, D], BF16, kind="ExternalOutput").ap()

    with tile.TileContext(nc) as tc:
        _emit(nc, tc, pk0a, pk0b, pk1a, pk2, pk1b, pk3, out, with_bq)
    nc.compile()
    return nc


def _emit(nc, tc, pk0a, pk0b, pk1a, pk2, pk1b, pk3, out, with_bq=False):
    WITH_BQ = with_bq
    from contextlib import ExitStack

    with ExitStack() as ctx:
        const = ctx.enter_context(tc.tile_pool(name="const", bufs=1))
        persist = ctx.enter_context(tc.tile_pool(name="persist", bufs=1))
        sqp = ctx.enter_context(tc.tile_pool(name="sq", bufs=2))
        ktp = ctx.enter_context(tc.tile_pool(name="ktp", bufs=2))
        vp = ctx.enter_context(tc.tile_pool(name="vp", bufs=2))
        esp = ctx.enter_context(tc.tile_pool(name="esp", bufs=2))
        estp = ctx.enter_context(tc.tile_pool(name="estp", bufs=2))
        sump = ctx.enter_context(tc.tile_pool(name="sump", bufs=2))
        otp = ctx.enter_context(tc.tile_pool(name="otp", bufs=2))
        outp = ctx.enter_context(tc.tile_pool(name="outp", bufs=2))
        psA = ctx.enter_context(tc.tile_pool(name="psA", bufs=4, space="PSUM"))
        psS = ctx.enter_context(tc.tile_pool(name="psS", bufs=2, space="PSUM"))
        psO = ctx.enter_context(tc.tile_pool(name="psO", bufs=2, space="PSUM"))

        # ---- packed SBUF landing tiles ----
        pk0a_sb = const.tile([P, 4096 + 4096], F8, name="pk0a_sb")
        pk0b_sb = const.tile([P, 4096], F8, name="pk0b_sb")
        pk1a_sb = const.tile([P, 8192 + 8192], F8, name="pk1a_sb")
        pk2_sb = const.tile([P, 8192 + 8192], F8, name="pk2_sb")
        pk1b_sb = const.tile([P, 8192], F8, name="pk1b_sb")
        pk3_sb = const.tile([P, 4096 + 8], BF16, name="pk3_sb")

        hT = pk0a_sb[:, 0:4096].rearrange("p (db i) -> p db i", db=DB)
        # wq split by output-column half so Q(hb 0-3) starts after 1MB
        wqA_v = pk0a_sb[:, 4096:8192].rearrange(
            "p (blk two h) -> p blk two h", blk=4, two=2)
        wqB_v = pk0b_sb[:, 0:4096].rearrange(
            "p (blk two h) -> p blk two h", blk=4, two=2)
        wk_v = pk1a_sb[:, 0:8192].rearrange(
            "p (blk two h) -> p blk two h", blk=4, two=2)
        eT01 = pk1a_sb[:, 8192:16384].rearrange(
            "p (pr db j) -> p pr db j", pr=2, db=DB)
        eT23 = pk1b_sb[:, 0:8192].rearrange(
            "p (pr db j) -> p pr db j", pr=2, db=DB)
        wv_v = pk2_sb[:, 0:8192].rearrange(
            "p (blk two h) -> p blk two h", blk=4, two=2)
        wo_v = pk2_sb[:, 8192:16384].rearrange(
            "p (blk two d) -> p blk two d", blk=4, two=2)
        hres_v = pk3_sb[:, 0:4096].rearrange("p (pr d) -> p pr d", pr=NPAIR)
        bq_sb = pk3_sb[:, 4096:4104]

        def eT(p):
            return eT01[:, p, :, :] if p < 2 else eT23[:, p - 2, :, :]

        ones = const.tile([P, 1], BF16, name="ones")
        ones_row = const.tile([1, P], BF16, name="ones_row")
        zeros = const.tile([P, 1], F32, name="zeros")
        epsc = const.tile([1, 1], F32, name="epsc")
        qT = persist.tile([P, HB, I], BF16, name="qT")

        kT = [None] * NPAIR
        v = [None] * NPAIR
        expS = [None] * NPAIR
        expST = [None] * NPAIR
        oT = [None] * NPAIR

        def emit_kt(p):
            # K^T [hd, j'] : weight blocks stationary
            kT[p] = ktp.tile([P, HB, JP], BF16, tag="kT", name=f"kT{p}")
            for hb in range(HB):
                ps_k = psA.tile([P, JP], F32, tag="A")
                for blk in range(4):
                    nc.tensor.matmul(
                        ps_k[:],
                        wk_v[:, blk, :, hb * P:(hb + 1) * P],
                        eT(p)[:, 2 * blk:2 * blk + 2, :],
                        start=(blk == 0),
                        stop=(blk == 3),
                        perf_mode=DR,
                    )
                nc.scalar.copy(kT[p][:, hb, :], ps_k[:])

        def emit_v(p):
            # V [j', hd] : e^T blocks stationary.  PSUM evacuation on DVE
            # mid-kernel (ScalarE queue backs up behind exp/oT ACTs at pair
            # boundaries); on ScalarE for the last pair (DVE runs the
            # endgame softmax chain there).
            v[p] = vp.tile([P, 4, HD], BF16, tag="v", name=f"v{p}")
            for jb in range(4):
                for half in range(2):
                    ps_v = psA.tile([P, 512], F32, tag="A")
                    for blk in range(4):
                        nc.tensor.matmul(
                            ps_v[:],
                            eT(p)[:, 2 * blk:2 * blk + 2, jb * P:(jb + 1) * P],
                            wv_v[:, blk, :, half * 512:(half + 1) * 512],
                            start=(blk == 0),
                            stop=(blk == 3),
                            perf_mode=DR,
                        )
                    nc.scalar.copy(
                        v[p][:, jb, half * 512:(half + 1) * 512], ps_v[:])

        def emit_S(p):
            # scores + exp; psS partition layout (hpar, i) so concurrent
            # row-group pairs write different output partitions (same-bank
            # same-partition concurrent PE writes are a HW fault).
            expS[p] = esp.tile([P, HB, JP], BF16, tag="expS", name=f"expS{p}")
            expST[p] = estp.tile([P, 4 * HB, P], BF16, tag="expST", name=f"expST{p}")
            sums = sump.tile([P, NH], F32, tag="sums")
            recip = sump.tile([P, NH], F32, tag="recip")
            for s in range(HB):  # head pair s -> heads 2s, 2s+1
                ps_s = psS.tile([P, 512], F32)
                for hpar in range(2):
                    for c01 in range(2):
                        nc.tensor.matmul(
                            ps_s[64 * hpar:64 * hpar + 64, 256 * c01:256 * c01 + 256],
                            qT[64 * hpar:64 * hpar + 64, s,
                               (2 * p + c01) * CL:(2 * p + c01) * CL + CL],
                            kT[p][64 * hpar:64 * hpar + 64, s,
                                  c01 * JC:(c01 + 1) * JC],
                            start=True, stop=True,
                        )
                nc.scalar.activation(
                    expS[p][:, s, :], ps_s[:], AF.Exp, bias=zeros[:],
                )
                nc.vector.reduce_sum(
                    sums[:, 2 * s:2 * s + 2],
                    expS[p][:, s, :].rearrange("p (c j) -> p c j", c=2),
                    axis=mybir.AxisListType.X)
                nc.vector.reciprocal_approx_fast(
                    recip[:, 2 * s:2 * s + 2], sums[:, 2 * s:2 * s + 2])
                nc.vector.tensor_mul(
                    expS[p][:, s, :].rearrange("p (c j) -> p c j", c=2),
                    expS[p][:, s, :].rearrange("p (c j) -> p c j", c=2),
                    recip[:, 2 * s:2 * s + 2].unsqueeze(-1).broadcast_to(
                        [P, 2, JC]))
                if s % 4 == 3:
                    # xbar transpose of the 4 finished s-tiles:
                    # out[pp, t, r] = attn[r, t*128+pp]
                    # alternate queues so a pair's two transposes overlap;
                    # last pair keeps both on sync (idle then) -- a scalar
                    # issue would queue behind the endgame exp/copy ACTs
                    eng = nc.sync if (s == 3 or p == NPAIR - 1) else nc.scalar
                    eng.dma_start(
                        out=expST[p][:, 4 * (s - 3):4 * (s - 3) + 16, :],
                        in_=expS[p][:, s - 3:s + 1, :].rearrange(
                            "p a b -> p (a b)"),
                        transpose=True,
                    )

        def emit_o(p):
            # o^T = attn @ V  (V slices stationary, attn^T streaming)
            oT[p] = otp.tile([P, HB, P], F8, tag="oT", name=f"oT{p}")
            for t2 in range(2):
                po = psO.tile([P, 512], F32)
                for c01 in range(2):
                    for kk in range(4):
                        for hpar in range(2):
                            s = 4 * t2 + kk
                            h = 2 * s + hpar
                            slot = c01 * 4 + kk
                            for jb in range(2):
                                nc.tensor.matmul(
                                    po[64 * hpar:64 * hpar + 64,
                                       64 * slot:64 * slot + 64],
                                    v[p][:, c01 * 2 + jb, h * DK:(h + 1) * DK],
                                    expST[p][:, 4 * s + 2 * c01 + jb,
                                             64 * hpar:64 * hpar + 64],
                                    start=(jb == 0), stop=(jb == 1),
                                )
                for c01 in range(2):
                    # mid-kernel oT copies on DVE (idle at pair boundaries;
                    # on ScalarE they block the kT/V psum evacuations the
                    # next pair's matmuls recycle against); last pair on
                    # ScalarE (DVE runs the endgame softmax chain)
                    dst = oT[p][:, 4 * t2:4 * t2 + 4, 64 * c01:64 * c01 + 64]
                    src = po[:, 256 * c01:256 * c01 + 256].rearrange(
                        "p (a b) -> p a b", a=4)
                    if p == NPAIR - 1:
                        nc.scalar.copy(dst, src)
                    else:
                        nc.vector.tensor_copy(dst, src)

        def emit_outproj(p):
            # out = oT^T @ wo + hres   (oT stationary fp8 DoubleRow)
            # last pair in quarter-column chunks so add+store pipeline and
            # the final store is small
            nchunk, w = (4, 256) if p == NPAIR - 1 else (2, 512)
            for ch in range(nchunk):
                ps_o = psA.tile([P, w], F32, tag="A")
                for blk in range(4):
                    nc.tensor.matmul(
                        ps_o[:],
                        oT[p][:, 2 * blk:2 * blk + 2, :],
                        wo_v[:, blk, :, ch * w:(ch + 1) * w],
                        start=(blk == 0),
                        stop=(blk == 3),
                        perf_mode=DR,
                    )
                osb = outp.tile([P, w], BF16, tag=f"osb{w}", bufs=4)
                nc.vector.tensor_add(
                    osb[:],
                    ps_o[:],
                    hres_v[:, p, ch * w:(ch + 1) * w],
                )
                # last pair's stores on the (idle by then) sync queue: the
                # gpsimd software DGE issues slowly and pays a ~2.5us drain
                eng = nc.sync if p == NPAIR - 1 else nc.gpsimd
                eng.dma_start(
                    out[p * P:(p + 1) * P, ch * w:(ch + 1) * w],
                    osb[:])

        # ---- prologue ----
        nc.vector.memset(ones[:], 1.0)
        nc.vector.memset(ones_row[:], 1.0)
        nc.vector.memset(zeros[:], 0.0)
        nc.vector.memset(epsc[:], EPS)
        # prewarm ScalarE LUTs (Exp/Sqrt/Square table loads ~1.3us on first use)
        warm = const.tile([1, 3], F32, name="warm")
        nc.scalar.activation(warm[:, 0:1], epsc[:], AF.Exp, bias=zeros[0:1, :])
        nc.scalar.activation(warm[:, 1:2], epsc[:], AF.Sqrt, bias=zeros[0:1, :])
        nc.scalar.activation(warm[:, 2:3], epsc[:], AF.Square, bias=zeros[0:1, :])
        # packed input DMAs, one per dram tensor, in consumption order
        nc.sync.dma_start(pk0a_sb[:], pk0a)
        nc.sync.dma_start(pk0b_sb[:], pk0b)
        nc.sync.dma_start(pk1a_sb[:], pk1a)
        nc.sync.dma_start(pk2_sb[:], pk2)
        nc.sync.dma_start(pk1b_sb[:], pk1b)
        nc.sync.dma_start(pk3_sb[:], pk3)

        # rms squares on ScalarE (DVE stays free for the qT muls later)
        sq = [None] * DB
        for db in range(DB):
            sq[db] = sqp.tile([P, I], BF16, tag="sq", bufs=8, name=f"sq{db}")
            nc.scalar.activation(sq[db][:], hT[:, db, :], AF.Square,
                                 bias=zeros[:])

        # ---- Q^T from raw hT; rmsnorm scale applied at the epilogue ----
        qTraw = persist.tile([P, HB, I], BF16, name="qTraw")
        for hb in range(HB):
            wq_v, hbo = (wqA_v, hb) if hb < 4 else (wqB_v, hb - 4)
            ps_q = psA.tile([P, I], F32, tag="A")
            for blk in range(4):
                nc.tensor.matmul(
                    ps_q[:],
                    wq_v[:, blk, :, hbo * P:(hbo + 1) * P],
                    hT[:, 2 * blk:2 * blk + 2, :],
                    start=(blk == 0),
                    stop=(blk == 3),
                    perf_mode=DR,
                )
            nc.scalar.copy(qTraw[:, hb, :], ps_q[:])
            if hb == 3:
                # rmsnorm stats early so rstd is ready well before S(0)
                ps_ss = psA.tile([1, I], F32, tag="A")
                for db in range(DB):
                    nc.tensor.matmul(
                        ps_ss[:], ones[:], sq[db][:],
                        start=(db == 0), stop=(db == DB - 1)
                    )

        ms = persist.tile([1, I], BF16, name="ms")
        nc.scalar.activation(ms[:], ps_ss[:], AF.Identity, bias=epsc[:],
                             scale=1.0 / D)
        # broadcast ms across partitions (PE outer product, K=1, bf16 so a
        # single matmul pass), then rsqrt at full width via
        # reciprocal_approx_fast + Sqrt activation.
        ps_msb = psA.tile([P, I], F32, tag="A")
        nc.tensor.matmul(ps_msb[:], ones_row[:], ms[:], start=True, stop=True)
        inv_msf = persist.tile([P, I], F32, name="inv_msf")
        nc.vector.reciprocal_approx_fast(inv_msf[:], ps_msb[:])
        rstd_full = persist.tile([P, I], F32, name="rstd_full")
        # rstd/8 in one shot: sqrt(inv_ms / 64) (folds the attention scale)
        nc.scalar.activation(rstd_full[:], inv_msf[:], AF.Sqrt, bias=zeros[:],
                             scale=1.0 / 64.0)
        for hb in range(HB):
            # qT = qTraw * rstd/8  (column-wise); bq added after if nonzero
            nc.vector.tensor_mul(qT[:, hb, :], qTraw[:, hb, :], rstd_full[:])
            if WITH_BQ:
                nc.scalar.activation(
                    qT[:, hb, :], qT[:, hb, :], AF.Identity,
                    bias=bq_sb[:, hb:hb + 1], scale=1.0,
                )

        # ---- software-pipelined pair loop ----
        # PE stream: Q, rms, KT0, V0, S0, KT1, V1, o0, S1, op0, KT2, V2, o1,
        # S2, op1, KT3, S3, o2, V3, op2, o3, op3.  S(p+1) is scheduled right
        # after KT(p+1); for the last pair S3 runs before o2/V3 so its softmax
        # + transpose chain is covered by ~12us of PE work.
        emit_kt(0)
        emit_v(0)
        emit_S(0)
        for p in range(NPAIR):
            if p + 1 < NPAIR:
                emit_kt(p + 1)
                if p + 1 == NPAIR - 1:
                    emit_S(p + 1)
                else:
                    emit_v(p + 1)
            emit_o(p)
            if p + 1 < NPAIR:
                if p + 1 == NPAIR - 1:
                    emit_v(p + 1)
                else:
                    emit_S(p + 1)
            emit_outproj(p)


def _get_nc(with_bq=False):
    if with_bq not in _CACHED:
        _CACHED[with_bq] = _build_nc(with_bq)
    return _CACHED[with_bq]


def _make_in_maps(h, e, g_norm, Wq, bq, Wk, bk, Wv, bv, Wo, bo):
    f8 = ml_dtypes.float8_e4m3
    bf = ml_dtypes.bfloat16
    h = np.asarray(h, np.float32)
    e = np.asarray(e, np.float32)
    # fold g_norm into Wq (rmsnorm gain only feeds the q projection)
    wq_f = np.asarray(g_norm, np.float32)[:, None] * np.asarray(Wq, np.float32)

    def wlayout(w):
        # [1024, X] -> [p, blk, two, X] flat, row = blk*256 + two*128 + p
        return np.ascontiguousarray(
            w.reshape(4, 2, 128, -1).transpose(2, 0, 1, 3).reshape(128, -1))

    wq8 = wlayout(wq_f.astype(f8))
    wqA = np.ascontiguousarray(
        wq8.reshape(128, 4, 2, 1024)[:, :, :, :512].reshape(128, 4096))
    wqB = np.ascontiguousarray(
        wq8.reshape(128, 4, 2, 1024)[:, :, :, 512:].reshape(128, 4096))
    wk8 = wlayout(np.asarray(Wk, np.float32).astype(f8))
    wv8 = wlayout(np.asarray(Wv, np.float32).astype(f8))
    wo8 = wlayout(np.asarray(Wo, np.float32).astype(f8))
    # bq applied on device (pre-scaled by attention scale); bk is a no-op
    # through softmax; bv/bo fold into the residual below.
    bqt = (np.asarray(bq, np.float32) / 8.0).reshape(HB, P).T.astype(bf)
    out_bias = None
    bv = np.asarray(bv, np.float32)
    bo = np.asarray(bo, np.float32)
    if np.any(bv) or np.any(bo):
        out_bias = bv @ np.asarray(Wo, np.float32) + bo

    in_maps = []
    meta = []
    for b in range(4):
        for g in range(2):
            start = 63 + 512 * g
            stop = min(1024, start + 512)
            nvalid = stop - start
            hs = np.zeros((512, D), np.float32)
            hs[:nvalid] = h[b, start:stop]
            hres = hs if out_bias is None else hs + out_bias[None, :]
            # hT [p, db, i] flat: row d = db*128 + p
            hT8 = hs.T.astype(f8).reshape(8, 128, 512).transpose(
                1, 0, 2).reshape(128, 4096)
            es = e[b, 8 * g:8 * (g + 1)].reshape(NCH * JC, D)
            # eT [p, pr, db, 512]: row d = db*128 + p, col j = pr*512 + c
            et = np.ascontiguousarray(
                es.T.astype(f8).reshape(8, 128, 4, 512).transpose(
                    1, 2, 0, 3).reshape(128, 16384))
            hres_p = np.ascontiguousarray(
                hres.reshape(4, 128, 1024).transpose(1, 0, 2).reshape(
                    128, 4096)).astype(bf)
            in_maps.append({
                "pk0a": np.concatenate([hT8, wqA], axis=1),
                "pk0b": wqB,
                "pk1a": np.concatenate([wk8, et[:, :8192]], axis=1),
                "pk2": np.concatenate([wv8, wo8], axis=1),
                "pk1b": np.ascontiguousarray(et[:, 8192:]),
                "pk3": np.concatenate([hres_p, bqt], axis=1),
            })
            meta.append((b, start, nvalid))
    return in_maps, meta


def _assemble(h, results, meta):
    outf = np.array(h, np.float32, copy=True)
    for core, (b, start, nvalid) in enumerate(meta):
        outf[b, start:start + nvalid] = results[core]["out"][:nvalid].astype(
            np.float32)
    # rows [0, 63) stay h (zero-padded attention output region)
    return outf


def kernel(h, e, g_norm, Wq, bq, Wk, bk, Wv, bv, Wo, bo):
    in_maps, meta = _make_in_maps(h, e, g_norm, Wq, bq, Wk, bk, Wv, bv, Wo, bo)
    nc = _get_nc(bool(np.any(np.asarray(bq))))
    res = run_bass_kernel_spmd(nc, in_maps, list(range(8)))
    return _assemble(h, res.results, meta)


def kernel_timed(trace=True, **inputs):
    """test-harness entry: returns (output, exec_time_ns)."""
    in_maps, meta = _make_in_maps(**inputs)
    nc = _get_nc(bool(np.any(np.asarray(inputs["bq"]))))
    res = run_bass_kernel_spmd(nc, in_maps, list(range(8)), trace=trace)
    return _assemble(inputs["h"], res.results, meta), res.exec_time_ns
